# revision 23
# baseline (speedup 1.0000x reference)
"""Causal self-attention (B=4, T=2048, C=1024, H=16) on 8 Trainium2 cores.

Sharding: data-parallel over batch (4 groups) x tensor-parallel over heads
(2-way). Core c = 2*b + t handles batch b, heads [t*8, t*8+8).

Per-core device kernel (all matmuls fp16 in / fp32 psum accumulate):
  - qkv projection computed in transposed layout: qk^T[feat, T] so that the
    per-head Q^T/K^T tiles feed the S^T matmul directly; V computed in
    natural [T, feat] layout (it is the stationary operand of the AV matmul)
    with a ones-column appended so the AV matmul also accumulates the
    softmax denominator l[q] for free.
  - Q/K live as HEAD PAIRS: q2[p]/k2[p] are [128, T] tiles with head 2p's
    64 dims on partitions 0-63 and head 2p+1's on 64-127. The two S^T
    matmuls of a pair then land on disjoint PE row-tiles ((0,0) and (64,0)
    via tile_position inference) and execute CONCURRENTLY on the PE --
    measured 2.7x faster than serial 64-contraction matmuls on this part.
  - S^T[k, q] = K^T.T @ Q^T per (pair, 128-wide k-block, 512-wide q-chunk),
    causal blocks only; exp via ACT (scale=1/8 folded in); additive -1e30
    mask on the 4 diagonal blocks of each q-chunk. AV for block j is
    emitted after the S-pair of block j+1 so the exp sits between the two
    on the scalar engine while the PE streams.
  - O^T_aug[65, q] = V_aug.T @ P^T accumulated over k-blocks; row 64 is l.
  - normalize via DVE reciprocal + partition-broadcast DMA.
  - output projection row-parallel: out^T = W_proj_half.T @ y^T (+ b_proj on
    the t=0 core); host sums the two partials per batch and transposes.
"""

import sys

import numpy as np

from concourse import mybir, tile, bacc
from concourse import bass_utils
from concourse.bass_utils import run_bass_kernel_spmd


def _ensure_trace_support():
    """Make trace=True / BASS_TRACE runs survive on images whose antenv lacks
    axon_hooks and where artifact upload has no credentials. Both shims are
    no-ops on the untraced path."""
    try:
        import antenv.axon_hooks  # noqa: F401
    except ImportError:
        import contextlib
        import ctypes
        import os
        import types

        mod = types.ModuleType("antenv.axon_hooks")
        state = {"hook": None, "tried": False}

        def set_axon_ntff_profile_hook(hook):
            state["hook"] = hook

        def _via_ctypes(so_path):
            lib = ctypes.CDLL(so_path)
            if not hasattr(lib, "axon_start_nrt_profile"):
                return None
            lib.axon_start_nrt_profile.argtypes = [
                ctypes.POINTER(ctypes.c_int64),
                ctypes.c_size_t,
            ]
            lib.axon_start_nrt_profile.restype = ctypes.c_int64
            lib.axon_stop_nrt_profile.argtypes = [ctypes.c_char_p]
            lib.axon_stop_nrt_profile.restype = ctypes.c_int64

            @contextlib.contextmanager
            def _hook(output_dir, device_ids):
                import jax

                jax.devices()
                if device_ids:
                    ids = (ctypes.c_int64 * len(device_ids))(*device_ids)
                    rc = lib.axon_start_nrt_profile(ids, len(device_ids))
                else:
                    rc = lib.axon_start_nrt_profile(None, 0)
                if rc != 0:
                    raise RuntimeError(f"axon_start_nrt_profile rc={rc}")
                try:
                    yield
                finally:
                    lib.axon_stop_nrt_profile(str(output_dir).encode())

            return _hook

        def get_axon_ntff_profile_hook():
            if state["hook"] is None and not state["tried"]:
                state["tried"] = True
                so = os.environ.get("AXON_PJRT_SO", "/opt/axon/libaxon_pjrt.so")
                if os.path.exists(so):
                    try:
                        state["hook"] = _via_ctypes(so)
                    except OSError:
                        pass
            return state["hook"]

        mod.set_axon_ntff_profile_hook = set_axon_ntff_profile_hook
        mod.get_axon_ntff_profile_hook = get_axon_ntff_profile_hook
        sys.modules["antenv.axon_hooks"] = mod

    orig_upload = bass_utils.upload_artifacts
    if not getattr(orig_upload, "_safe_wrapped", False):
        def _safe_upload(tmpdir):
            try:
                return orig_upload(tmpdir)
            except Exception:
                return "local://" + str(tmpdir)

        _safe_upload._safe_wrapped = True
        bass_utils.upload_artifacts = _safe_upload


_ensure_trace_support()

F16 = mybir.dt.float16
F32 = mybir.dt.float32
EXPF = mybir.ActivationFunctionType.Exp
IDF = mybir.ActivationFunctionType.Identity

B, T, C, H, D = 4, 2048, 1024, 16, 64
HPC = 8          # heads per core
QC = 512         # q-chunk width
NT = T // 128    # 16 T-tiles of 128
NQC = T // QC    # 4 q-chunks
NKC = C // 128   # 8 contraction tiles for the input projections
NKP = (HPC * D) // 128  # 4 contraction tiles for the output projection
NEG = -1.0e30

_CACHE = {}


def _build():
    nc = bacc.Bacc("TRN2", target_bir_lowering=False, debug=False)

    xT = nc.dram_tensor("xT", [C, T], F16, kind="ExternalInput")
    wqk = nc.dram_tensor("wqk", [C, HPC * 128], F16, kind="ExternalInput")
    bqk = nc.dram_tensor("bqk", [HPC * 128], F32, kind="ExternalInput")
    wv = nc.dram_tensor("wv", [C, HPC * D], F16, kind="ExternalInput")
    bv = nc.dram_tensor("bv", [HPC * D], F16, kind="ExternalInput")
    wp = nc.dram_tensor("wp", [HPC * D, C], F16, kind="ExternalInput")
    bp = nc.dram_tensor("bp", [C], F32, kind="ExternalInput")
    mask = nc.dram_tensor("mask", [4, 128, QC], F32, kind="ExternalInput")
    outT = nc.dram_tensor("outT", [C, T], F32, kind="ExternalOutput")

    with tile.TileContext(nc) as tc:
        with (
            tc.tile_pool(name="wts", bufs=1) as wpool,
            tc.tile_pool(name="qk", bufs=1) as qkpool,
            tc.tile_pool(name="vy", bufs=1) as vypool,
            tc.tile_pool(name="xc", bufs=2) as xpool,
            tc.tile_pool(name="pt", bufs=6) as ppool,
            tc.tile_pool(name="tmp", bufs=4) as tpool,
            tc.tile_pool(name="st", bufs=4) as spool,
            tc.tile_pool(name="sm", bufs=4) as smallpool,
            tc.tile_pool(name="ot", bufs=2) as otpool,
            tc.tile_pool(name="dr", bufs=2, space="DRAM") as drpool,
            tc.tile_pool(name="blk", bufs=5, space="PSUM") as pspool,
            tc.tile_pool(name="av", bufs=3, space="PSUM") as opool,
        ):
            # ---- weights / constants ----
            # DMA order matters at kernel start: the first x chunk and wv
            # come first so the V projection can start ~5us in; wqk streams
            # in 8 per-m-tile slices consumed in order; everything needed
            # later (mask, wp, biases) loads behind them.
            wv_sb = wpool.tile([128, NKC, HPC * D], F16, tag="wv")
            wv_re = wv.ap().rearrange("(a p) m -> p a m", p=128)
            # V bias as a partition-broadcast tile: added on the DVE during
            # the PSUM->SBUF copy (saves a 512-col PE matmul per T-tile)
            bv_sb = wpool.tile([128, HPC * D], F16, tag="bv")
            nc.sync.dma_start(
                bv_sb[:],
                bv.ap().rearrange("(o n) -> o n", o=1).partition_broadcast(128),
            )
            ones64 = wpool.tile([1, 64], F16, tag="ones64")
            nc.vector.memset(ones64[:], 1.0)
            # (wqk / bqk / mask / wp / bp DMAs are emitted inside chunk 0,
            # behind the first x-chunk DMA, so the V projection starts as
            # early as possible)

            # ---- persistent activation tiles ----
            # head-pair layout: pair p holds head 2p on partitions 0-63 and
            # head 2p+1 on partitions 64-127
            q2 = [qkpool.tile([128, T], F16, tag=f"q{p}", name=f"q{p}") for p in range(HPC // 2)]
            k2 = [qkpool.tile([128, T], F16, tag=f"k{p}", name=f"k{p}") for p in range(HPC // 2)]
            # V with ones column: [128 part, T-tile, head, 64+1]
            v_sb = vypool.tile([128, NT, HPC, D + 1], F16, tag="v")
            nc.vector.memset(v_sb[:, :, :, D : D + 1], 1.0)
            y_t = [vypool.tile([128, T], F16, tag=f"y{g}", name=f"y{g}") for g in range(NKP)]

            # Softmax normalization runs entirely off the PE: per head the
            # denominator row l goes to a DRAM scratch and O is copied to
            # SBUF f16 (freeing the PSUM bank). One chunk's 8 l-rows are
            # then reloaded as [128, 32] so a single DVE reciprocal (cost
            # is proportional to the free dim) covers the whole chunk; each
            # head's 1/l row is partition-broadcast back via DMA and y is a
            # cheap f16 x f16 DVE multiply. The flush is deferred past the
            # next chunk's input projections so every dependency is long
            # resolved (a >3.4us PE gap would re-throttle it to 1.2 GHz).
            def emit_flush_pe(qc, ots, lrows):
                # Normalize via SBUF-only chain: DVE reciprocal per head row,
                # K=1 ones-matmul partition-broadcast (tiny PE cost),
                # DVE multiply. No DRAM round trip, so y(qc) is ready a few
                # microseconds after the last ot copy and the out-projection
                # can interleave into the next chunk's attention.
                q0 = qc * QC
                for (h, ot), lr in zip(ots, lrows):
                    rr = smallpool.tile(
                        [1, QC], F32, tag=f"rr{h % 4}", name="rr", bufs=1
                    )
                    nc.vector.reciprocal(rr[:], lr[:])
                    rr16 = smallpool.tile(
                        [1, QC], F16, tag=f"rs{h % 4}", name="rr16", bufs=1
                    )
                    nc.vector.tensor_copy(rr16[:], rr[:])
                    bcs_ps = pspool.tile([64, QC], F32, tag="blk", name="bcsps")
                    nc.tensor.matmul(
                        bcs_ps[:], ones64[0:1, :], rr16[:], start=True, stop=True
                    )
                    r0 = (h % 2) * 64
                    nc.vector.tensor_mul(
                        y_t[h // 2][r0 : r0 + 64, q0 : q0 + QC], ot[:], bcs_ps[:]
                    )

            def emit_proj(qc, ms):
                q0 = qc * QC
                for m in ms:
                    pps = pspool.tile([128, QC], F32, tag="blk", name="pps")
                    for kk in range(NKP):
                        nc.tensor.matmul(
                            pps[:],
                            wp_sb[:, kk, m * 128 : (m + 1) * 128],
                            y_t[kk][:, q0 : q0 + QC],
                            start=(kk == 0),
                            stop=(kk == NKP - 1),
                        )
                    st = spool.tile([128, QC], F32, tag="st", name="st")
                    # bias add on the DVE: the scalar engine is the
                    # bottleneck during attention (exp), where these now run
                    nc.vector.tensor_scalar_add(st[:], pps[:], bp_sb[:, m : m + 1])
                    # stores go on the SWDGE (gpsimd) queue so they never
                    # delay latency-critical loads/broadcasts on the HWDGE
                    nc.gpsimd.dma_start(
                        outT.ap()[m * 128 : (m + 1) * 128, q0 : q0 + QC], st[:]
                    )

            # out-proj m-tiles of chunk qc-1 woven into attention(qc): pair 0
            # has none (flush chain still resolving), later pairs carry 3/3/2
            PROJ_SCHED = {0: [], 1: [0, 1, 2], 2: [3, 4, 5], 3: [6, 7]}

            pending = None  # (qc, [(h, ot_sb)...], [lrow...]) awaiting flush

            for qc in range(NQC):
                q0 = qc * QC
                # ---- input projections for this T-chunk ----
                xc = xpool.tile([128, NKC, QC], F16, tag="xc")
                xc_re = xT.ap()[:, q0 : q0 + QC].rearrange("(a p) n -> p a n", p=128)
                if qc == 0:
                    # interleave x / wv per contraction tile so the first
                    # V-proj matmul can start after ~260KB instead of 2MB
                    for kk in range(NKC):
                        nc.sync.dma_start(xc[:, kk], xc_re[:, kk])
                        nc.sync.dma_start(wv_sb[:, kk], wv_re[:, kk])
                else:
                    nc.sync.dma_start(xc[:], xc_re)
                # V projection first: at qc=0 it only needs xc + wv, which
                # are the first two DMAs issued.
                for tt in range(4):
                    tg = qc * 4 + tt
                    ps = pspool.tile([128, QC], F32, tag="blk")
                    for kk in range(NKC):
                        nc.tensor.matmul(
                            ps[:],
                            xc[:, kk, tt * 128 : (tt + 1) * 128],
                            wv_sb[:, kk, :],
                            start=(kk == 0),
                            stop=(kk == NKC - 1),
                        )
                    nc.vector.tensor_add(
                        v_sb[:, tg, :, 0:D],
                        ps[:].rearrange("p (h d) -> p h d", d=D),
                        bv_sb[:].rearrange("p (h d) -> p h d", d=D),
                    )
                if qc == 0:
                    # remaining weights stream behind xc0/wv on the DMA queue
                    wqk_sb = wpool.tile([128, NKC, HPC * 128], F16, tag="wqk")
                    nc.sync.dma_start(
                        wqk_sb[:], wqk.ap().rearrange("(a p) m -> p a m", p=128)
                    )
                    bqk_sb = wpool.tile([128, HPC], F32, tag="bqk")
                    nc.sync.dma_start(
                        bqk_sb[:], bqk.ap().rearrange("(m p) -> p m", p=128)
                    )
                    mask_sb = wpool.tile([128, 4, QC], F32, tag="mask")
                    nc.sync.dma_start(mask_sb[:], mask.ap().rearrange("o p n -> p o n"))
                    wp_sb = wpool.tile([128, NKP, C], F16, tag="wp")
                    nc.sync.dma_start(
                        wp_sb[:], wp.ap().rearrange("(a p) m -> p a m", p=128)
                    )
                    bp_sb = wpool.tile([128, C // 128], F32, tag="bp")
                    nc.sync.dma_start(bp_sb[:], bp.ap().rearrange("(m p) -> p m", p=128))
                # m-tile 2p is the Q-pair for pair p, 2p+1 the K-pair (host
                # packs wqk accordingly), so one full-partition DVE add moves
                # each PSUM tile into its persistent pair tile.
                for m in range(HPC):
                    ps = pspool.tile([128, QC], F32, tag="blk")
                    for kk in range(NKC):
                        nc.tensor.matmul(
                            ps[:],
                            wqk_sb[:, kk, m * 128 : (m + 1) * 128],
                            xc[:, kk, :],
                            start=(kk == 0),
                            stop=(kk == NKC - 1),
                        )
                    dst = q2[m // 2] if m % 2 == 0 else k2[m // 2]
                    nc.vector.tensor_scalar_add(
                        dst[:, q0 : q0 + QC], ps[:], bqk_sb[:, m : m + 1]
                    )

                # Flush the previous chunk's normalization now (the input
                # projections above gave the PE ~25us of cover); its
                # out-projection m-tiles are woven into the attention pair
                # loop below so they fill PE stalls while the scalar engine
                # works through the exps.
                if pending is not None:
                    emit_flush_pe(pending[0], pending[1], pending[2])

                # ---- attention for q-chunk qc, head pairs ----
                # Per k-block: the pair's two S^T matmuls issue back-to-back
                # on disjoint PE row tiles ((0,0)/(64,0)) and run
                # concurrently; AV of block j-1 is emitted after the S-pair
                # of block j so its exp has a full S-pair of PE time to
                # complete on the scalar engine.
                n_k = qc * 4 + 4
                ots = []
                lrows = []
                for p in range(HPC // 2):
                    if pending is not None and PROJ_SCHED[p]:
                        emit_proj(qc - 1, PROJ_SCHED[p])
                    ha, hb = 2 * p, 2 * p + 1
                    ops_a = opool.tile([128, QC], F32, tag="av", name="opsa")
                    ops_b = opool.tile([128, QC], F32, tag="av", name="opsb")
                    pend_av = None  # (j, c0, w, pt_a, pt_b)
                    for j in range(n_k):
                        o = j - (n_k - 4)  # diagonal offset, <0 for full blocks
                        c0 = 128 * o if o > 0 else 0  # cols < c0 fully masked
                        w = QC - c0
                        sps_a = pspool.tile([128, QC], F32, tag="blk", name="spsa")
                        sps_b = pspool.tile([128, QC], F32, tag="blk", name="spsb")
                        nc.tensor.matmul(
                            sps_a[:, 0:w],
                            k2[p][0:64, j * 128 : (j + 1) * 128],
                            q2[p][0:64, q0 + c0 : q0 + QC],
                            start=True,
                            stop=True,
                        )
                        nc.tensor.matmul(
                            sps_b[:, 0:w],
                            k2[p][64:128, j * 128 : (j + 1) * 128],
                            q2[p][64:128, q0 + c0 : q0 + QC],
                            start=True,
                            stop=True,
                        )
                        pt_a = ppool.tile([128, QC], F16, tag="pt", name="pta")
                        pt_b = ppool.tile([128, QC], F16, tag="pt", name="ptb")
                        for sps, pt in ((sps_a, pt_a), (sps_b, pt_b)):
                            if o >= 0:
                                mt = tpool.tile([128, QC], F32, tag="mt", name="mt")
                                nc.vector.tensor_add(
                                    mt[:, 0:w], sps[:, 0:w], mask_sb[:, o, c0:QC]
                                )
                                nc.scalar.activation(
                                    pt[:, 0:w], mt[:, 0:w], EXPF, scale=0.125
                                )
                            else:
                                nc.scalar.activation(
                                    pt[:, 0:w], sps[:, 0:w], EXPF, scale=0.125
                                )
                        if pend_av is not None:
                            jj, cc0, ww, qa, qb = pend_av
                            nc.tensor.matmul(
                                ops_a[0:65, cc0:QC],
                                v_sb[:, jj, ha, :],
                                qa[:, 0:ww],
                                start=(jj == 0),
                                stop=False,
                            )
                            nc.tensor.matmul(
                                ops_b[0:65, cc0:QC],
                                v_sb[:, jj, hb, :],
                                qb[:, 0:ww],
                                start=(jj == 0),
                                stop=False,
                            )
                        pend_av = (j, c0, w, pt_a, pt_b)
                    jj, cc0, ww, qa, qb = pend_av
                    nc.tensor.matmul(
                        ops_a[0:65, cc0:QC],
                        v_sb[:, jj, ha, :],
                        qa[:, 0:ww],
                        start=(jj == 0),
                        stop=True,
                    )
                    nc.tensor.matmul(
                        ops_b[0:65, cc0:QC],
                        v_sb[:, jj, hb, :],
                        qb[:, 0:ww],
                        start=(jj == 0),
                        stop=True,
                    )
                    for h, ops in ((ha, ops_a), (hb, ops_b)):
                        # denominator row and O to SBUF (frees the PSUM bank
                        # for the next pair); lrows feed the next chunk's
                        # SBUF-only flush
                        lrow = smallpool.tile(
                            [1, QC], F32, tag=f"lrT{h}", name="lrow", bufs=1
                        )
                        nc.vector.tensor_copy(lrow[:], ops[64:65, :])
                        lrows.append(lrow)
                        ot = otpool.tile([64, QC], F16, tag=f"ot{h}", name=f"ot{h}")
                        nc.vector.tensor_copy(ot[:], ops[0:64, :])
                        ots.append((h, ot))
                pending = (qc, ots, lrows)

            # tail: flush the last chunk (SBUF-only chain), then its
            # out-projection
            emit_flush_pe(pending[0], pending[1], pending[2])
            emit_proj(NQC - 1, list(range(C // 128)))

    nc.compile()
    return nc


def _shards(W_attn, b_attn, W_proj, b_proj):
    """Per-TP-half weight shards (t = 0, 1), packed for the device layout."""
    shards = []
    for t in range(2):
        heads = list(range(t * HPC, (t + 1) * HPC))
        wqk = np.empty((C, HPC * 128), np.float16)
        bqk = np.empty(HPC * 128, np.float32)
        wv = np.empty((C, HPC * D), np.float16)
        bvv = np.empty(HPC * D, np.float16)
        # m-tile 2p = Q of head pair p (head 2p on cols 0-63 of the tile,
        # head 2p+1 on 64-127); m-tile 2p+1 = K of the same pair.
        for p in range(HPC // 2):
            ha, hb = heads[2 * p], heads[2 * p + 1]
            mq, mk = (2 * p) * 128, (2 * p + 1) * 128
            wqk[:, mq : mq + 64] = W_attn[:, ha * D : (ha + 1) * D]
            wqk[:, mq + 64 : mq + 128] = W_attn[:, hb * D : (hb + 1) * D]
            wqk[:, mk : mk + 64] = W_attn[:, C + ha * D : C + (ha + 1) * D]
            wqk[:, mk + 64 : mk + 128] = W_attn[:, C + hb * D : C + (hb + 1) * D]
            bqk[mq : mq + 64] = b_attn[ha * D : (ha + 1) * D]
            bqk[mq + 64 : mq + 128] = b_attn[hb * D : (hb + 1) * D]
            bqk[mk : mk + 64] = b_attn[C + ha * D : C + (ha + 1) * D]
            bqk[mk + 64 : mk + 128] = b_attn[C + hb * D : C + (hb + 1) * D]
        for j, h in enumerate(heads):
            wv[:, j * D : (j + 1) * D] = W_attn[:, 2 * C + h * D : 2 * C + (h + 1) * D]
            bvv[j * D : (j + 1) * D] = b_attn[2 * C + h * D : 2 * C + (h + 1) * D]
        wp = W_proj[t * HPC * D : (t + 1) * HPC * D, :].astype(np.float16)
        bpp = (b_proj if t == 0 else np.zeros_like(b_proj)).astype(np.float32)
        shards.append((wqk, bqk, wv, bvv, np.ascontiguousarray(wp), bpp))
    return shards


def _mask_np():
    kr = np.arange(128)[:, None]
    qr = np.arange(QC)[None, :]
    m = np.empty((4, 128, QC), np.float32)
    for o in range(4):
        m[o] = np.where(kr + o * 128 <= qr, 0.0, NEG)
    return m


def _in_maps(x, W_attn, b_attn, W_proj, b_proj):
    shards = _shards(W_attn, b_attn, W_proj, b_proj)
    mask = _mask_np()
    in_maps = []
    for b in range(B):
        xTb = np.ascontiguousarray(x[b].T.astype(np.float16))
        for t in range(2):
            wqk, bqk, wv, bvv, wp, bpp = shards[t]
            in_maps.append(
                {
                    "xT": xTb,
                    "wqk": wqk,
                    "bqk": bqk,
                    "wv": wv,
                    "bv": bvv,
                    "wp": wp,
                    "bp": bpp,
                    "mask": mask,
                }
            )
    return in_maps


def _gather(results):
    out = np.empty((B, T, C), np.float32)
    for b in range(B):
        acc = results[2 * b]["outT"] + results[2 * b + 1]["outT"]
        out[b] = acc.T
    return out


def kernel(x, W_attn, b_attn, W_proj, b_proj):
    x = np.asarray(x, np.float32)
    W_attn = np.asarray(W_attn, np.float32)
    b_attn = np.asarray(b_attn, np.float32)
    W_proj = np.asarray(W_proj, np.float32)
    b_proj = np.asarray(b_proj, np.float32)

    if "nc" not in _CACHE:
        _CACHE["nc"] = _build()
    nc = _CACHE["nc"]

    in_maps = _in_maps(x, W_attn, b_attn, W_proj, b_proj)
    res = run_bass_kernel_spmd(nc, in_maps, core_ids=list(range(8)))
    return _gather(res.results)



# revision 27
# speedup vs baseline: 1.1397x; 1.1397x over previous
"""Causal self-attention (B=4, T=2048, C=1024, H=16) on 8 Trainium2 cores.

Sharding: data-parallel over batch (4 groups) x tensor-parallel over heads
(2-way). Core c = 2*b + t handles batch b, heads [t*8, t*8+8).

Per-core device kernel (all matmuls fp16 in / fp32 psum accumulate):
  - qkv projection computed in transposed layout: qk^T[feat, T] so that the
    per-head Q^T/K^T tiles feed the S^T matmul directly; V computed in
    natural [T, feat] layout (it is the stationary operand of the AV matmul)
    with a ones-column appended so the AV matmul also accumulates the
    softmax denominator l[q] for free.
  - Q/K live as HEAD PAIRS: q2[p]/k2[p] are [128, T] tiles with head 2p's
    64 dims on partitions 0-63 and head 2p+1's on 64-127. The two S^T
    matmuls of a pair then land on disjoint PE row-tiles ((0,0) and (64,0)
    via tile_position inference) and execute CONCURRENTLY on the PE --
    measured 2.7x faster than serial 64-contraction matmuls on this part.
  - S^T[k, q] = K^T.T @ Q^T per (pair, 128-wide k-block, 512-wide q-chunk),
    causal blocks only; exp via ACT (scale=1/8 folded in); additive -1e30
    mask on the 4 diagonal blocks of each q-chunk. AV for block j is
    emitted after the S-pair of block j+1 so the exp sits between the two
    on the scalar engine while the PE streams.
  - O^T_aug[65, q] = V_aug.T @ P^T accumulated over k-blocks; row 64 is l.
  - normalize via DVE reciprocal + partition-broadcast DMA.
  - output projection row-parallel: out^T = W_proj_half.T @ y^T (+ b_proj on
    the t=0 core); host sums the two partials per batch and transposes.
"""

import sys

import numpy as np

from concourse import mybir, tile, bacc
from concourse import bass_utils
from concourse.bass_utils import run_bass_kernel_spmd


def _ensure_trace_support():
    """Make trace=True / BASS_TRACE runs survive on images whose antenv lacks
    axon_hooks and where artifact upload has no credentials. Both shims are
    no-ops on the untraced path."""
    try:
        import antenv.axon_hooks  # noqa: F401
    except ImportError:
        import contextlib
        import ctypes
        import os
        import types

        mod = types.ModuleType("antenv.axon_hooks")
        state = {"hook": None, "tried": False}

        def set_axon_ntff_profile_hook(hook):
            state["hook"] = hook

        def _via_ctypes(so_path):
            lib = ctypes.CDLL(so_path)
            if not hasattr(lib, "axon_start_nrt_profile"):
                return None
            lib.axon_start_nrt_profile.argtypes = [
                ctypes.POINTER(ctypes.c_int64),
                ctypes.c_size_t,
            ]
            lib.axon_start_nrt_profile.restype = ctypes.c_int64
            lib.axon_stop_nrt_profile.argtypes = [ctypes.c_char_p]
            lib.axon_stop_nrt_profile.restype = ctypes.c_int64

            @contextlib.contextmanager
            def _hook(output_dir, device_ids):
                import jax

                jax.devices()
                if device_ids:
                    ids = (ctypes.c_int64 * len(device_ids))(*device_ids)
                    rc = lib.axon_start_nrt_profile(ids, len(device_ids))
                else:
                    rc = lib.axon_start_nrt_profile(None, 0)
                if rc != 0:
                    raise RuntimeError(f"axon_start_nrt_profile rc={rc}")
                try:
                    yield
                finally:
                    lib.axon_stop_nrt_profile(str(output_dir).encode())

            return _hook

        def get_axon_ntff_profile_hook():
            if state["hook"] is None and not state["tried"]:
                state["tried"] = True
                so = os.environ.get("AXON_PJRT_SO", "/opt/axon/libaxon_pjrt.so")
                if os.path.exists(so):
                    try:
                        state["hook"] = _via_ctypes(so)
                    except OSError:
                        pass
            return state["hook"]

        mod.set_axon_ntff_profile_hook = set_axon_ntff_profile_hook
        mod.get_axon_ntff_profile_hook = get_axon_ntff_profile_hook
        sys.modules["antenv.axon_hooks"] = mod

    orig_upload = bass_utils.upload_artifacts
    if not getattr(orig_upload, "_safe_wrapped", False):
        def _safe_upload(tmpdir):
            try:
                return orig_upload(tmpdir)
            except Exception:
                return "local://" + str(tmpdir)

        _safe_upload._safe_wrapped = True
        bass_utils.upload_artifacts = _safe_upload


_ensure_trace_support()

F16 = mybir.dt.float16
F32 = mybir.dt.float32
EXPF = mybir.ActivationFunctionType.Exp
IDF = mybir.ActivationFunctionType.Identity

B, T, C, H, D = 4, 2048, 1024, 16, 64
HPC = 8          # heads per core
QC = 512         # q-chunk width
NT = T // 128    # 16 T-tiles of 128
NQC = T // QC    # 4 q-chunks
NKC = C // 128   # 8 contraction tiles for the input projections
NKP = (HPC * D) // 128  # 4 contraction tiles for the output projection
NEG = -1.0e30

_CACHE = {}


def _build():
    nc = bacc.Bacc("TRN2", target_bir_lowering=False, debug=False)

    xT = nc.dram_tensor("xT", [C, T], F16, kind="ExternalInput")
    wqk = nc.dram_tensor("wqk", [C, HPC * 128], F16, kind="ExternalInput")
    bqk = nc.dram_tensor("bqk", [HPC * 128], F32, kind="ExternalInput")
    wv = nc.dram_tensor("wv", [C, HPC * D], F16, kind="ExternalInput")
    bv = nc.dram_tensor("bv", [HPC * D], F16, kind="ExternalInput")
    wp = nc.dram_tensor("wp", [HPC * D, C], F16, kind="ExternalInput")
    bp = nc.dram_tensor("bp", [C], F32, kind="ExternalInput")
    mask = nc.dram_tensor("mask", [4, 128, QC], F32, kind="ExternalInput")
    outT = nc.dram_tensor("outT", [C, T], F32, kind="ExternalOutput")

    with tile.TileContext(nc) as tc:
        with (
            tc.tile_pool(name="wts", bufs=1) as wpool,
            tc.tile_pool(name="qk", bufs=1) as qkpool,
            tc.tile_pool(name="vy", bufs=1) as vypool,
            tc.tile_pool(name="xc", bufs=2) as xpool,
            tc.tile_pool(name="pt", bufs=6) as ppool,
            tc.tile_pool(name="tmp", bufs=4) as tpool,
            tc.tile_pool(name="st", bufs=4) as spool,
            tc.tile_pool(name="sm", bufs=4) as smallpool,
            tc.tile_pool(name="ot", bufs=2) as otpool,
            tc.tile_pool(name="dr", bufs=2, space="DRAM") as drpool,
            tc.tile_pool(name="blk", bufs=5, space="PSUM") as pspool,
            tc.tile_pool(name="av", bufs=3, space="PSUM") as opool,
        ):
            # ---- weights / constants ----
            # DMA order matters at kernel start: the first x chunk and wv
            # come first so the V projection can start ~5us in; wqk streams
            # in 8 per-m-tile slices consumed in order; everything needed
            # later (mask, wp, biases) loads behind them.
            wv_sb = wpool.tile([128, NKC, HPC * D], F16, tag="wv")
            wv_re = wv.ap().rearrange("(a p) m -> p a m", p=128)
            # V bias as a partition-broadcast tile: added on the DVE during
            # the PSUM->SBUF copy (saves a 512-col PE matmul per T-tile)
            bv_sb = wpool.tile([128, HPC * D], F16, tag="bv")
            nc.sync.dma_start(
                bv_sb[:],
                bv.ap().rearrange("(o n) -> o n", o=1).partition_broadcast(128),
            )
            ones64 = wpool.tile([1, 64], F16, tag="ones64")
            nc.vector.memset(ones64[:], 1.0)
            # (wqk / bqk / mask / wp / bp DMAs are emitted inside chunk 0,
            # behind the first x-chunk DMA, so the V projection starts as
            # early as possible)

            # ---- persistent activation tiles ----
            # head-pair layout: pair p holds head 2p on partitions 0-63 and
            # head 2p+1 on partitions 64-127
            q2 = [qkpool.tile([128, T], F16, tag=f"q{p}", name=f"q{p}") for p in range(HPC // 2)]
            k2 = [qkpool.tile([128, T], F16, tag=f"k{p}", name=f"k{p}") for p in range(HPC // 2)]
            # V with ones column: [128 part, T-tile, head, 64+1]
            v_sb = vypool.tile([128, NT, HPC, D + 1], F16, tag="v")
            nc.vector.memset(v_sb[:, :, :, D : D + 1], 1.0)
            y_t = [vypool.tile([128, T], F16, tag=f"y{g}", name=f"y{g}") for g in range(NKP)]

            # Softmax normalization runs entirely off the PE: per head the
            # denominator row l goes to a DRAM scratch and O is copied to
            # SBUF f16 (freeing the PSUM bank). One chunk's 8 l-rows are
            # then reloaded as [128, 32] so a single DVE reciprocal (cost
            # is proportional to the free dim) covers the whole chunk; each
            # head's 1/l row is partition-broadcast back via DMA and y is a
            # cheap f16 x f16 DVE multiply. The flush is deferred past the
            # next chunk's input projections so every dependency is long
            # resolved (a >3.4us PE gap would re-throttle it to 1.2 GHz).
            def emit_flush_pe(qc, ots, lrows):
                # Normalize via SBUF-only chain: DVE reciprocal per head row,
                # K=1 ones-matmul partition-broadcast (tiny PE cost),
                # DVE multiply. No DRAM round trip, so y(qc) is ready a few
                # microseconds after the last ot copy and the out-projection
                # can interleave into the next chunk's attention.
                q0 = qc * QC
                for (h, ot), lr in zip(ots, lrows):
                    rr = smallpool.tile(
                        [1, QC], F32, tag=f"rr{h % 4}", name="rr", bufs=1
                    )
                    nc.vector.reciprocal(rr[:], lr[:])
                    rr16 = smallpool.tile(
                        [1, QC], F16, tag=f"rs{h % 4}", name="rr16", bufs=1
                    )
                    nc.vector.tensor_copy(rr16[:], rr[:])
                    bcs_ps = pspool.tile([64, QC], F32, tag="blk", name="bcsps")
                    nc.tensor.matmul(
                        bcs_ps[:], ones64[0:1, :], rr16[:], start=True, stop=True
                    )
                    r0 = (h % 2) * 64
                    nc.vector.tensor_mul(
                        y_t[h // 2][r0 : r0 + 64, q0 : q0 + QC], ot[:], bcs_ps[:]
                    )

            def emit_flush(qc, ots, lrows, l_dram):
                # Mid-chunk variant: everything off the PE (its queue is
                # in-order, so any PE instruction waiting on this chain
                # would stall the attention matmuls behind it). One chunk's
                # 8 l-rows reload as [128, 32] so a single DVE reciprocal
                # covers the chunk; 1/l partition-broadcasts back via DMA.
                q0 = qc * QC
                lall = smallpool.tile([128, 32], F32, tag="lall", name="lall")
                nc.sync.dma_start(
                    lall[:],
                    l_dram[:]
                    .rearrange("a (p n) -> (a p) n", n=32)
                    .rearrange("(a p) n -> p a n", p=128),
                )
                rall = smallpool.tile([128, 32], F32, tag="rall", name="rall")
                nc.vector.reciprocal(rall[:], lall[:])
                r16 = smallpool.tile([128, 32], F16, tag="r16", name="r16")
                nc.vector.tensor_copy(r16[:], rall[:])
                r16_dram = drpool.tile([HPC, QC], F16, tag="rdram", name="rdram")
                nc.sync.dma_start(
                    r16_dram[:]
                    .rearrange("a (p n) -> (a p) n", n=32)
                    .rearrange("(a p) n -> p a n", p=128),
                    r16[:],
                )
                for h, ot in ots:
                    bcs = tpool.tile([64, QC], F16, tag="bcs", name="bcs")
                    nc.sync.dma_start(
                        bcs[:], r16_dram[h : h + 1, :].partition_broadcast(64)
                    )
                    r0 = (h % 2) * 64
                    nc.vector.tensor_mul(
                        y_t[h // 2][r0 : r0 + 64, q0 : q0 + QC], ot[:], bcs[:]
                    )

            def emit_proj(qc, ms):
                q0 = qc * QC
                for m in ms:
                    pps = pspool.tile([128, QC], F32, tag="blk", name="pps")
                    for kk in range(NKP):
                        nc.tensor.matmul(
                            pps[:],
                            wp_sb[:, kk, m * 128 : (m + 1) * 128],
                            y_t[kk][:, q0 : q0 + QC],
                            start=(kk == 0),
                            stop=(kk == NKP - 1),
                        )
                    st = spool.tile([128, QC], F32, tag="st", name="st")
                    nc.scalar.activation(st[:], pps[:], IDF, bias=bp_sb[:, m : m + 1])
                    # stores go on the SWDGE (gpsimd) queue so they never
                    # delay latency-critical loads/broadcasts on the HWDGE
                    nc.gpsimd.dma_start(
                        outT.ap()[m * 128 : (m + 1) * 128, q0 : q0 + QC], st[:]
                    )

            pending = None  # (qc, ots, lrows, l_dram) awaiting flush

            for qc in range(NQC):
                q0 = qc * QC
                # ---- input projections for this T-chunk ----
                xc = xpool.tile([128, NKC, QC], F16, tag="xc")
                xc_re = xT.ap()[:, q0 : q0 + QC].rearrange("(a p) n -> p a n", p=128)
                if qc == 0:
                    # interleave x / wv per contraction tile so the first
                    # V-proj matmul can start after ~260KB instead of 2MB
                    for kk in range(NKC):
                        nc.sync.dma_start(xc[:, kk], xc_re[:, kk])
                        nc.sync.dma_start(wv_sb[:, kk], wv_re[:, kk])
                else:
                    nc.sync.dma_start(xc[:], xc_re)
                # V projection first: at qc=0 it only needs xc + wv, which
                # are the first two DMAs issued.
                for tt in range(4):
                    tg = qc * 4 + tt
                    ps = pspool.tile([128, QC], F32, tag="blk")
                    for kk in range(NKC):
                        nc.tensor.matmul(
                            ps[:],
                            xc[:, kk, tt * 128 : (tt + 1) * 128],
                            wv_sb[:, kk, :],
                            start=(kk == 0),
                            stop=(kk == NKC - 1),
                        )
                    nc.vector.tensor_add(
                        v_sb[:, tg, :, 0:D],
                        ps[:].rearrange("p (h d) -> p h d", d=D),
                        bv_sb[:].rearrange("p (h d) -> p h d", d=D),
                    )
                if qc == 0:
                    # remaining weights stream behind xc0/wv on the DMA queue
                    wqk_sb = wpool.tile([128, NKC, HPC * 128], F16, tag="wqk")
                    nc.sync.dma_start(
                        wqk_sb[:], wqk.ap().rearrange("(a p) m -> p a m", p=128)
                    )
                    bqk_sb = wpool.tile([128, HPC], F32, tag="bqk")
                    nc.sync.dma_start(
                        bqk_sb[:], bqk.ap().rearrange("(m p) -> p m", p=128)
                    )
                    mask_sb = wpool.tile([128, 4, QC], F32, tag="mask")
                    nc.sync.dma_start(mask_sb[:], mask.ap().rearrange("o p n -> p o n"))
                    wp_sb = wpool.tile([128, NKP, C], F16, tag="wp")
                    nc.sync.dma_start(
                        wp_sb[:], wp.ap().rearrange("(a p) m -> p a m", p=128)
                    )
                    bp_sb = wpool.tile([128, C // 128], F32, tag="bp")
                    nc.sync.dma_start(bp_sb[:], bp.ap().rearrange("(m p) -> p m", p=128))
                # m-tile 2p is the Q-pair for pair p, 2p+1 the K-pair (host
                # packs wqk accordingly), so one full-partition DVE add moves
                # each PSUM tile into its persistent pair tile.
                for m in range(HPC):
                    ps = pspool.tile([128, QC], F32, tag="blk")
                    for kk in range(NKC):
                        nc.tensor.matmul(
                            ps[:],
                            wqk_sb[:, kk, m * 128 : (m + 1) * 128],
                            xc[:, kk, :],
                            start=(kk == 0),
                            stop=(kk == NKC - 1),
                        )
                    dst = q2[m // 2] if m % 2 == 0 else k2[m // 2]
                    nc.vector.tensor_scalar_add(
                        dst[:, q0 : q0 + QC], ps[:], bqk_sb[:, m : m + 1]
                    )

                # Flush the previous chunk's normalization now (the input
                # projections above gave the PE ~25us of cover), then emit
                # the previous chunk's output projection. proj(2) is held
                # back until after attention(3) so it covers the tail
                # flush(3) chain.
                if pending is not None:
                    emit_flush(pending[0], pending[1], pending[2], pending[3])
                    if qc - 1 < NQC - 2:
                        emit_proj(qc - 1, list(range(C // 128)))
                pending = None

                # ---- attention for q-chunk qc, head pairs ----
                # Per k-block: the pair's two S^T matmuls issue back-to-back
                # on disjoint PE row tiles ((0,0)/(64,0)) and run
                # concurrently; AV of block j-1 is emitted after the S-pair
                # of block j so its exp has a full S-pair of PE time to
                # complete on the scalar engine.
                n_k = qc * 4 + 4
                l_dram = drpool.tile([HPC, QC], F32, tag="ldram", name="ldram")
                ots = []
                lrows = []
                tail = qc == NQC - 1
                for p in range(HPC // 2):
                    ha, hb = 2 * p, 2 * p + 1
                    ops_a = opool.tile([128, QC], F32, tag="av", name="opsa")
                    ops_b = opool.tile([128, QC], F32, tag="av", name="opsb")
                    pend_av = None  # (j, c0, w, pt_a, pt_b)
                    for j in range(n_k):
                        o = j - (n_k - 4)  # diagonal offset, <0 for full blocks
                        c0 = 128 * o if o > 0 else 0  # cols < c0 fully masked
                        w = QC - c0
                        sps_a = pspool.tile([128, QC], F32, tag="blk", name="spsa")
                        sps_b = pspool.tile([128, QC], F32, tag="blk", name="spsb")
                        nc.tensor.matmul(
                            sps_a[:, 0:w],
                            k2[p][0:64, j * 128 : (j + 1) * 128],
                            q2[p][0:64, q0 + c0 : q0 + QC],
                            start=True,
                            stop=True,
                        )
                        nc.tensor.matmul(
                            sps_b[:, 0:w],
                            k2[p][64:128, j * 128 : (j + 1) * 128],
                            q2[p][64:128, q0 + c0 : q0 + QC],
                            start=True,
                            stop=True,
                        )
                        pt_a = ppool.tile([128, QC], F16, tag="pt", name="pta")
                        pt_b = ppool.tile([128, QC], F16, tag="pt", name="ptb")
                        for sps, pt in ((sps_a, pt_a), (sps_b, pt_b)):
                            if o >= 0:
                                mt = tpool.tile([128, QC], F32, tag="mt", name="mt")
                                nc.vector.tensor_add(
                                    mt[:, 0:w], sps[:, 0:w], mask_sb[:, o, c0:QC]
                                )
                                nc.scalar.activation(
                                    pt[:, 0:w], mt[:, 0:w], EXPF, scale=0.125
                                )
                            else:
                                nc.scalar.activation(
                                    pt[:, 0:w], sps[:, 0:w], EXPF, scale=0.125
                                )
                        if pend_av is not None:
                            jj, cc0, ww, qa, qb = pend_av
                            nc.tensor.matmul(
                                ops_a[0:65, cc0:QC],
                                v_sb[:, jj, ha, :],
                                qa[:, 0:ww],
                                start=(jj == 0),
                                stop=False,
                            )
                            nc.tensor.matmul(
                                ops_b[0:65, cc0:QC],
                                v_sb[:, jj, hb, :],
                                qb[:, 0:ww],
                                start=(jj == 0),
                                stop=False,
                            )
                        pend_av = (j, c0, w, pt_a, pt_b)
                    jj, cc0, ww, qa, qb = pend_av
                    nc.tensor.matmul(
                        ops_a[0:65, cc0:QC],
                        v_sb[:, jj, ha, :],
                        qa[:, 0:ww],
                        start=(jj == 0),
                        stop=True,
                    )
                    nc.tensor.matmul(
                        ops_b[0:65, cc0:QC],
                        v_sb[:, jj, hb, :],
                        qb[:, 0:ww],
                        start=(jj == 0),
                        stop=True,
                    )
                    for h, ops in ((ha, ops_a), (hb, ops_b)):
                        # denominator row to DRAM scratch (DMA cannot read
                        # PSUM, so hop through SBUF); O to SBUF f16 (frees
                        # the PSUM bank for the next pair). The tail chunk
                        # keeps its lrows in SBUF for the PE-broadcast flush.
                        lrow = smallpool.tile(
                            [1, QC], F32,
                            tag=f"lrT{h}" if tail else "lrow", name="lrow",
                            bufs=1 if tail else None,
                        )
                        nc.vector.tensor_copy(lrow[:], ops[64:65, :])
                        if not tail:
                            nc.sync.dma_start(l_dram[h : h + 1, :], lrow[:])
                        lrows.append(lrow)
                        ot = otpool.tile([64, QC], F16, tag=f"ot{h}", name=f"ot{h}")
                        nc.vector.tensor_copy(ot[:], ops[0:64, :])
                        ots.append((h, ot))
                pending = (qc, ots, lrows, l_dram)

            # tail: the held-back proj(2) gives the PE ~13us of work while
            # the last chunk's flush chain (DVE + K=1 broadcast) resolves,
            # then proj(3)
            emit_proj(NQC - 2, list(range(C // 128)))
            emit_flush_pe(pending[0], pending[1], pending[2])
            emit_proj(NQC - 1, list(range(C // 128)))

    nc.compile()
    return nc


def _shards(W_attn, b_attn, W_proj, b_proj):
    """Per-TP-half weight shards (t = 0, 1), packed for the device layout."""
    shards = []
    for t in range(2):
        heads = list(range(t * HPC, (t + 1) * HPC))
        wqk = np.empty((C, HPC * 128), np.float16)
        bqk = np.empty(HPC * 128, np.float32)
        wv = np.empty((C, HPC * D), np.float16)
        bvv = np.empty(HPC * D, np.float16)
        # m-tile 2p = Q of head pair p (head 2p on cols 0-63 of the tile,
        # head 2p+1 on 64-127); m-tile 2p+1 = K of the same pair.
        for p in range(HPC // 2):
            ha, hb = heads[2 * p], heads[2 * p + 1]
            mq, mk = (2 * p) * 128, (2 * p + 1) * 128
            wqk[:, mq : mq + 64] = W_attn[:, ha * D : (ha + 1) * D]
            wqk[:, mq + 64 : mq + 128] = W_attn[:, hb * D : (hb + 1) * D]
            wqk[:, mk : mk + 64] = W_attn[:, C + ha * D : C + (ha + 1) * D]
            wqk[:, mk + 64 : mk + 128] = W_attn[:, C + hb * D : C + (hb + 1) * D]
            bqk[mq : mq + 64] = b_attn[ha * D : (ha + 1) * D]
            bqk[mq + 64 : mq + 128] = b_attn[hb * D : (hb + 1) * D]
            bqk[mk : mk + 64] = b_attn[C + ha * D : C + (ha + 1) * D]
            bqk[mk + 64 : mk + 128] = b_attn[C + hb * D : C + (hb + 1) * D]
        for j, h in enumerate(heads):
            wv[:, j * D : (j + 1) * D] = W_attn[:, 2 * C + h * D : 2 * C + (h + 1) * D]
            bvv[j * D : (j + 1) * D] = b_attn[2 * C + h * D : 2 * C + (h + 1) * D]
        wp = W_proj[t * HPC * D : (t + 1) * HPC * D, :].astype(np.float16)
        bpp = (b_proj if t == 0 else np.zeros_like(b_proj)).astype(np.float32)
        shards.append((wqk, bqk, wv, bvv, np.ascontiguousarray(wp), bpp))
    return shards


def _mask_np():
    kr = np.arange(128)[:, None]
    qr = np.arange(QC)[None, :]
    m = np.empty((4, 128, QC), np.float32)
    for o in range(4):
        m[o] = np.where(kr + o * 128 <= qr, 0.0, NEG)
    return m


def _in_maps(x, W_attn, b_attn, W_proj, b_proj):
    shards = _shards(W_attn, b_attn, W_proj, b_proj)
    mask = _mask_np()
    in_maps = []
    for b in range(B):
        xTb = np.ascontiguousarray(x[b].T.astype(np.float16))
        for t in range(2):
            wqk, bqk, wv, bvv, wp, bpp = shards[t]
            in_maps.append(
                {
                    "xT": xTb,
                    "wqk": wqk,
                    "bqk": bqk,
                    "wv": wv,
                    "bv": bvv,
                    "wp": wp,
                    "bp": bpp,
                    "mask": mask,
                }
            )
    return in_maps


def _gather(results):
    out = np.empty((B, T, C), np.float32)
    for b in range(B):
        acc = results[2 * b]["outT"] + results[2 * b + 1]["outT"]
        out[b] = acc.T
    return out


def kernel(x, W_attn, b_attn, W_proj, b_proj):
    x = np.asarray(x, np.float32)
    W_attn = np.asarray(W_attn, np.float32)
    b_attn = np.asarray(b_attn, np.float32)
    W_proj = np.asarray(W_proj, np.float32)
    b_proj = np.asarray(b_proj, np.float32)

    if "nc" not in _CACHE:
        _CACHE["nc"] = _build()
    nc = _CACHE["nc"]

    in_maps = _in_maps(x, W_attn, b_attn, W_proj, b_proj)
    res = run_bass_kernel_spmd(nc, in_maps, core_ids=list(range(8)))
    return _gather(res.results)



# revision 29
# speedup vs baseline: 1.4501x; 1.2724x over previous
"""Causal self-attention (B=4, T=2048, C=1024, H=16) on 8 Trainium2 cores.

Sharding: data-parallel over batch (4 groups) x tensor-parallel over heads
(2-way). Core c = 2*b + t handles batch b, heads [t*8, t*8+8).

Per-core device kernel (all matmuls fp16 in / fp32 psum accumulate):
  - qkv projection computed in transposed layout: qk^T[feat, T] so that the
    per-head Q^T/K^T tiles feed the S^T matmul directly; V computed in
    natural [T, feat] layout (it is the stationary operand of the AV matmul)
    with a ones-column appended so the AV matmul also accumulates the
    softmax denominator l[q] for free.
  - Q/K live as HEAD PAIRS: q2[p]/k2[p] are [128, T] tiles with head 2p's
    64 dims on partitions 0-63 and head 2p+1's on 64-127. The two S^T
    matmuls of a pair then land on disjoint PE row-tiles ((0,0) and (64,0)
    via tile_position inference) and execute CONCURRENTLY on the PE --
    measured 2.7x faster than serial 64-contraction matmuls on this part.
  - S^T[k, q] = K^T.T @ Q^T per (pair, 128-wide k-block, 512-wide q-chunk),
    causal blocks only; exp via ACT (scale=1/8 folded in); additive -1e30
    mask on the 4 diagonal blocks of each q-chunk. AV for block j is
    emitted after the S-pair of block j+1 so the exp sits between the two
    on the scalar engine while the PE streams.
  - O^T_aug[65, q] = V_aug.T @ P^T accumulated over k-blocks; row 64 is l.
  - normalize via DVE reciprocal + partition-broadcast DMA.
  - output projection row-parallel: out^T = W_proj_half.T @ y^T (+ b_proj on
    the t=0 core); host sums the two partials per batch and transposes.
"""

import sys

import numpy as np

from concourse import mybir, tile, bacc
from concourse import bass_utils
from concourse.bass_utils import run_bass_kernel_spmd


def _ensure_trace_support():
    """Make trace=True / BASS_TRACE runs survive on images whose antenv lacks
    axon_hooks and where artifact upload has no credentials. Both shims are
    no-ops on the untraced path."""
    try:
        import antenv.axon_hooks  # noqa: F401
    except ImportError:
        import contextlib
        import ctypes
        import os
        import types

        mod = types.ModuleType("antenv.axon_hooks")
        state = {"hook": None, "tried": False}

        def set_axon_ntff_profile_hook(hook):
            state["hook"] = hook

        def _via_ctypes(so_path):
            lib = ctypes.CDLL(so_path)
            if not hasattr(lib, "axon_start_nrt_profile"):
                return None
            lib.axon_start_nrt_profile.argtypes = [
                ctypes.POINTER(ctypes.c_int64),
                ctypes.c_size_t,
            ]
            lib.axon_start_nrt_profile.restype = ctypes.c_int64
            lib.axon_stop_nrt_profile.argtypes = [ctypes.c_char_p]
            lib.axon_stop_nrt_profile.restype = ctypes.c_int64

            @contextlib.contextmanager
            def _hook(output_dir, device_ids):
                import jax

                jax.devices()
                if device_ids:
                    ids = (ctypes.c_int64 * len(device_ids))(*device_ids)
                    rc = lib.axon_start_nrt_profile(ids, len(device_ids))
                else:
                    rc = lib.axon_start_nrt_profile(None, 0)
                if rc != 0:
                    raise RuntimeError(f"axon_start_nrt_profile rc={rc}")
                try:
                    yield
                finally:
                    lib.axon_stop_nrt_profile(str(output_dir).encode())

            return _hook

        def get_axon_ntff_profile_hook():
            if state["hook"] is None and not state["tried"]:
                state["tried"] = True
                so = os.environ.get("AXON_PJRT_SO", "/opt/axon/libaxon_pjrt.so")
                if os.path.exists(so):
                    try:
                        state["hook"] = _via_ctypes(so)
                    except OSError:
                        pass
            return state["hook"]

        mod.set_axon_ntff_profile_hook = set_axon_ntff_profile_hook
        mod.get_axon_ntff_profile_hook = get_axon_ntff_profile_hook
        sys.modules["antenv.axon_hooks"] = mod

    orig_upload = bass_utils.upload_artifacts
    if not getattr(orig_upload, "_safe_wrapped", False):
        def _safe_upload(tmpdir):
            try:
                return orig_upload(tmpdir)
            except Exception:
                return "local://" + str(tmpdir)

        _safe_upload._safe_wrapped = True
        bass_utils.upload_artifacts = _safe_upload


_ensure_trace_support()

F16 = mybir.dt.float16
F32 = mybir.dt.float32
EXPF = mybir.ActivationFunctionType.Exp
IDF = mybir.ActivationFunctionType.Identity

B, T, C, H, D = 4, 2048, 1024, 16, 64
HPC = 8          # heads per core
QC = 512         # q-chunk width
NT = T // 128    # 16 T-tiles of 128
NQC = T // QC    # 4 q-chunks
NKC = C // 128   # 8 contraction tiles for the input projections
NKP = (HPC * D) // 128  # 4 contraction tiles for the output projection
NEG = -1.0e30

_CACHE = {}


def _build():
    nc = bacc.Bacc("TRN2", target_bir_lowering=False, debug=False)

    xT = nc.dram_tensor("xT", [C, T], F16, kind="ExternalInput")
    wqk = nc.dram_tensor("wqk", [C, HPC * 128], F16, kind="ExternalInput")
    bqk = nc.dram_tensor("bqk", [HPC * 128], F32, kind="ExternalInput")
    wv = nc.dram_tensor("wv", [C, HPC * D], F16, kind="ExternalInput")
    bv = nc.dram_tensor("bv", [HPC * D], F16, kind="ExternalInput")
    wp = nc.dram_tensor("wp", [HPC * D, C], F16, kind="ExternalInput")
    bp = nc.dram_tensor("bp", [C], F32, kind="ExternalInput")
    mask = nc.dram_tensor("mask", [4, 128, QC], F32, kind="ExternalInput")
    outT = nc.dram_tensor("outT", [C, T], F32, kind="ExternalOutput")

    with tile.TileContext(nc) as tc:
        with (
            tc.tile_pool(name="wts", bufs=1) as wpool,
            tc.tile_pool(name="qk", bufs=1) as qkpool,
            tc.tile_pool(name="vy", bufs=1) as vypool,
            tc.tile_pool(name="xc", bufs=2) as xpool,
            tc.tile_pool(name="pt", bufs=6) as ppool,
            tc.tile_pool(name="tmp", bufs=4) as tpool,
            tc.tile_pool(name="st", bufs=4) as spool,
            tc.tile_pool(name="sm", bufs=4) as smallpool,
            tc.tile_pool(name="ot", bufs=2) as otpool,
            tc.tile_pool(name="dr", bufs=2, space="DRAM") as drpool,
            tc.tile_pool(name="blk", bufs=5, space="PSUM") as pspool,
            tc.tile_pool(name="av", bufs=3, space="PSUM") as opool,
        ):
            # ---- weights / constants ----
            # DMA order matters at kernel start: the first x chunk and wv
            # come first so the V projection can start ~5us in; wqk streams
            # in 8 per-m-tile slices consumed in order; everything needed
            # later (mask, wp, biases) loads behind them.
            wv_sb = wpool.tile([128, NKC, HPC * D], F16, tag="wv")
            wv_re = wv.ap().rearrange("(a p) m -> p a m", p=128)
            # V bias as a partition-broadcast tile: added on the DVE during
            # the PSUM->SBUF copy (saves a 512-col PE matmul per T-tile)
            bv_sb = wpool.tile([128, HPC * D], F16, tag="bv")
            nc.sync.dma_start(
                bv_sb[:],
                bv.ap().rearrange("(o n) -> o n", o=1).partition_broadcast(128),
            )
            ones64 = wpool.tile([1, 64], F16, tag="ones64")
            nc.vector.memset(ones64[:], 1.0)
            # (wqk / bqk / mask / wp / bp DMAs are emitted inside chunk 0,
            # behind the first x-chunk DMA, so the V projection starts as
            # early as possible)

            # ---- persistent activation tiles ----
            # head-pair layout: pair p holds head 2p on partitions 0-63 and
            # head 2p+1 on partitions 64-127
            q2 = [qkpool.tile([128, T], F16, tag=f"q{p}", name=f"q{p}") for p in range(HPC // 2)]
            k2 = [qkpool.tile([128, T], F16, tag=f"k{p}", name=f"k{p}") for p in range(HPC // 2)]
            # V with ones column: [128 part, T-tile, head, 64+1]
            v_sb = vypool.tile([128, NT, HPC, D + 1], F16, tag="v")
            nc.vector.memset(v_sb[:, :, :, D : D + 1], 1.0)
            y_t = [vypool.tile([128, T], F16, tag=f"y{g}", name=f"y{g}") for g in range(NKP)]

            # Softmax normalization runs entirely off the PE: per head the
            # denominator row l goes to a DRAM scratch and O is copied to
            # SBUF f16 (freeing the PSUM bank). One chunk's 8 l-rows are
            # then reloaded as [128, 32] so a single DVE reciprocal (cost
            # is proportional to the free dim) covers the whole chunk; each
            # head's 1/l row is partition-broadcast back via DMA and y is a
            # cheap f16 x f16 DVE multiply. The flush is deferred past the
            # next chunk's input projections so every dependency is long
            # resolved (a >3.4us PE gap would re-throttle it to 1.2 GHz).
            def emit_flush_pe(qc, ots, lrows):
                # Normalize via SBUF-only chain: DVE reciprocal per head row,
                # K=1 ones-matmul partition-broadcast (tiny PE cost),
                # DVE multiply. No DRAM round trip, so y(qc) is ready a few
                # microseconds after the last ot copy and the out-projection
                # can interleave into the next chunk's attention.
                q0 = qc * QC
                for (h, ot), lr in zip(ots, lrows):
                    rr = smallpool.tile(
                        [1, QC], F32, tag=f"rr{h % 4}", name="rr", bufs=1
                    )
                    nc.vector.reciprocal(rr[:], lr[:])
                    rr16 = smallpool.tile(
                        [1, QC], F16, tag=f"rs{h % 4}", name="rr16", bufs=1
                    )
                    nc.vector.tensor_copy(rr16[:], rr[:])
                    bcs_ps = pspool.tile([64, QC], F32, tag="blk", name="bcsps")
                    nc.tensor.matmul(
                        bcs_ps[:], ones64[0:1, :], rr16[:], start=True, stop=True
                    )
                    r0 = (h % 2) * 64
                    nc.vector.tensor_mul(
                        y_t[h // 2][r0 : r0 + 64, q0 : q0 + QC], ot[:], bcs_ps[:]
                    )

            def emit_flush(qc, ots, lrows, l_dram):
                # Mid-chunk variant: everything off the PE (its queue is
                # in-order, so any PE instruction waiting on this chain
                # would stall the attention matmuls behind it). One chunk's
                # 8 l-rows reload as [128, 32] so a single DVE reciprocal
                # covers the chunk; 1/l partition-broadcasts back via DMA.
                q0 = qc * QC
                lall = smallpool.tile([128, 32], F32, tag="lall", name="lall")
                nc.sync.dma_start(
                    lall[:],
                    l_dram[:]
                    .rearrange("a (p n) -> (a p) n", n=32)
                    .rearrange("(a p) n -> p a n", p=128),
                )
                rall = smallpool.tile([128, 32], F32, tag="rall", name="rall")
                nc.vector.reciprocal(rall[:], lall[:])
                r16 = smallpool.tile([128, 32], F16, tag="r16", name="r16")
                nc.vector.tensor_copy(r16[:], rall[:])
                r16_dram = drpool.tile([HPC, QC], F16, tag="rdram", name="rdram")
                nc.sync.dma_start(
                    r16_dram[:]
                    .rearrange("a (p n) -> (a p) n", n=32)
                    .rearrange("(a p) n -> p a n", p=128),
                    r16[:],
                )
                for h, ot in ots:
                    bcs = tpool.tile([64, QC], F16, tag="bcs", name="bcs")
                    nc.sync.dma_start(
                        bcs[:], r16_dram[h : h + 1, :].partition_broadcast(64)
                    )
                    r0 = (h % 2) * 64
                    nc.vector.tensor_mul(
                        y_t[h // 2][r0 : r0 + 64, q0 : q0 + QC], ot[:], bcs[:]
                    )

            def emit_proj(qc, ms):
                q0 = qc * QC
                for m in ms:
                    pps = pspool.tile([128, QC], F32, tag="blk", name="pps")
                    for kk in range(NKP):
                        nc.tensor.matmul(
                            pps[:],
                            wp_sb[:, kk, m * 128 : (m + 1) * 128],
                            y_t[kk][:, q0 : q0 + QC],
                            start=(kk == 0),
                            stop=(kk == NKP - 1),
                        )
                    st = spool.tile([128, QC], F32, tag="st", name="st")
                    nc.scalar.activation(st[:], pps[:], IDF, bias=bp_sb[:, m : m + 1])
                    # stores go on the SWDGE (gpsimd) queue so they never
                    # delay latency-critical loads/broadcasts on the HWDGE
                    nc.gpsimd.dma_start(
                        outT.ap()[m * 128 : (m + 1) * 128, q0 : q0 + QC], st[:]
                    )

            pending = None  # (qc, ots, lrows, l_dram) awaiting flush

            for qc in range(NQC):
                q0 = qc * QC
                # ---- input projections for this T-chunk ----
                xc = xpool.tile([128, NKC, QC], F16, tag="xc")
                xc_re = xT.ap()[:, q0 : q0 + QC].rearrange("(a p) n -> p a n", p=128)
                if qc == 0:
                    # interleave x / wv per contraction tile so the first
                    # V-proj matmul can start after ~260KB instead of 2MB
                    for kk in range(NKC):
                        nc.sync.dma_start(xc[:, kk], xc_re[:, kk])
                        nc.sync.dma_start(wv_sb[:, kk], wv_re[:, kk])
                else:
                    nc.sync.dma_start(xc[:], xc_re)
                # V projection first: at qc=0 it only needs xc + wv, which
                # are the first two DMAs issued.
                for tt in range(4):
                    tg = qc * 4 + tt
                    ps = pspool.tile([128, QC], F32, tag="blk")
                    for kk in range(NKC):
                        nc.tensor.matmul(
                            ps[:],
                            xc[:, kk, tt * 128 : (tt + 1) * 128],
                            wv_sb[:, kk, :],
                            start=(kk == 0),
                            stop=(kk == NKC - 1),
                        )
                    nc.vector.tensor_add(
                        v_sb[:, tg, :, 0:D],
                        ps[:].rearrange("p (h d) -> p h d", d=D),
                        bv_sb[:].rearrange("p (h d) -> p h d", d=D),
                    )
                if qc == 0:
                    # remaining weights stream behind xc0/wv on the DMA queue
                    wqk_sb = wpool.tile([128, NKC, HPC * 128], F16, tag="wqk")
                    nc.sync.dma_start(
                        wqk_sb[:], wqk.ap().rearrange("(a p) m -> p a m", p=128)
                    )
                    bqk_sb = wpool.tile([128, HPC], F32, tag="bqk")
                    nc.sync.dma_start(
                        bqk_sb[:], bqk.ap().rearrange("(m p) -> p m", p=128)
                    )
                    mask_sb = wpool.tile([128, 4, QC], F32, tag="mask")
                    nc.sync.dma_start(mask_sb[:], mask.ap().rearrange("o p n -> p o n"))
                    wp_sb = wpool.tile([128, NKP, C], F16, tag="wp")
                    nc.sync.dma_start(
                        wp_sb[:], wp.ap().rearrange("(a p) m -> p a m", p=128)
                    )
                    bp_sb = wpool.tile([128, C // 128], F32, tag="bp")
                    nc.sync.dma_start(bp_sb[:], bp.ap().rearrange("(m p) -> p m", p=128))
                # m-tile 2p is the Q-pair for pair p, 2p+1 the K-pair (host
                # packs wqk accordingly), so one full-partition DVE add moves
                # each PSUM tile into its persistent pair tile.
                for m in range(HPC):
                    ps = pspool.tile([128, QC], F32, tag="blk")
                    for kk in range(NKC):
                        nc.tensor.matmul(
                            ps[:],
                            wqk_sb[:, kk, m * 128 : (m + 1) * 128],
                            xc[:, kk, :],
                            start=(kk == 0),
                            stop=(kk == NKC - 1),
                        )
                    dst = q2[m // 2] if m % 2 == 0 else k2[m // 2]
                    nc.vector.tensor_scalar_add(
                        dst[:, q0 : q0 + QC], ps[:], bqk_sb[:, m : m + 1]
                    )

                # Flush the previous chunk's normalization now (the input
                # projections above gave the PE ~25us of cover), then emit
                # the previous chunk's output projection. proj(2) is held
                # back until after attention(3) so it covers the tail
                # flush(3) chain.
                if pending is not None:
                    emit_flush(pending[0], pending[1], pending[2], pending[3])
                    if qc - 1 < NQC - 2:
                        emit_proj(qc - 1, list(range(C // 128)))
                pending = None

                # ---- attention for q-chunk qc, head pairs ----
                # Per k-block: the pair's two S^T matmuls issue back-to-back
                # on disjoint PE row tiles ((0,0)/(64,0)) and run
                # concurrently; AV of block j-1 is emitted after the S-pair
                # of block j so its exp has a full S-pair of PE time to
                # complete on the scalar engine.
                n_k = qc * 4 + 4
                l_dram = drpool.tile([HPC, QC], F32, tag="ldram", name="ldram")
                ots = []
                lrows = []
                tail = qc == NQC - 1
                for p in range(HPC // 2):
                    ha, hb = 2 * p, 2 * p + 1
                    ops_a = opool.tile([128, QC], F32, tag="av", name="opsa")
                    ops_b = opool.tile([128, QC], F32, tag="av", name="opsb")
                    pend_av = []  # [(j, c0, w, pt_a, pt_b)] AV lag queue

                    def emit_av(entry, last):
                        jj, cc0, ww, qa, qb = entry
                        nc.tensor.matmul(
                            ops_a[0:65, cc0:QC],
                            v_sb[:, jj, ha, :],
                            qa[:, 0:ww],
                            start=(jj == 0),
                            stop=last,
                        )
                        nc.tensor.matmul(
                            ops_b[0:65, cc0:QC],
                            v_sb[:, jj, hb, :],
                            qb[:, 0:ww],
                            start=(jj == 0),
                            stop=last,
                        )
                    for j in range(n_k):
                        o = j - (n_k - 4)  # diagonal offset, <0 for full blocks
                        c0 = 128 * o if o > 0 else 0  # cols < c0 fully masked
                        w = QC - c0
                        sps_a = pspool.tile([128, QC], F32, tag="blk", name="spsa")
                        sps_b = pspool.tile([128, QC], F32, tag="blk", name="spsb")
                        nc.tensor.matmul(
                            sps_a[:, 0:w],
                            k2[p][0:64, j * 128 : (j + 1) * 128],
                            q2[p][0:64, q0 + c0 : q0 + QC],
                            start=True,
                            stop=True,
                        )
                        nc.tensor.matmul(
                            sps_b[:, 0:w],
                            k2[p][64:128, j * 128 : (j + 1) * 128],
                            q2[p][64:128, q0 + c0 : q0 + QC],
                            start=True,
                            stop=True,
                        )
                        pt_a = ppool.tile([128, QC], F16, tag="pt", name="pta")
                        pt_b = ppool.tile([128, QC], F16, tag="pt", name="ptb")
                        for sps, pt in ((sps_a, pt_a), (sps_b, pt_b)):
                            if o >= 0:
                                mt = tpool.tile([128, QC], F32, tag="mt", name="mt")
                                nc.vector.tensor_add(
                                    mt[:, 0:w], sps[:, 0:w], mask_sb[:, o, c0:QC]
                                )
                                nc.scalar.activation(
                                    pt[:, 0:w], mt[:, 0:w], EXPF, scale=0.125
                                )
                            else:
                                nc.scalar.activation(
                                    pt[:, 0:w], sps[:, 0:w], EXPF, scale=0.125
                                )
                        if len(pend_av) == 2:
                            emit_av(pend_av.pop(0), last=False)
                        pend_av.append((j, c0, w, pt_a, pt_b))
                    while pend_av:
                        emit_av(pend_av.pop(0), last=not pend_av)
                    for h, ops in ((ha, ops_a), (hb, ops_b)):
                        # denominator row to DRAM scratch (DMA cannot read
                        # PSUM, so hop through SBUF); O to SBUF f16 (frees
                        # the PSUM bank for the next pair). The tail chunk
                        # keeps its lrows in SBUF for the PE-broadcast flush.
                        lrow = smallpool.tile(
                            [1, QC], F32,
                            tag=f"lrT{h}" if tail else "lrow", name="lrow",
                            bufs=1 if tail else None,
                        )
                        nc.vector.tensor_copy(lrow[:], ops[64:65, :])
                        if not tail:
                            nc.sync.dma_start(l_dram[h : h + 1, :], lrow[:])
                        lrows.append(lrow)
                        ot = otpool.tile([64, QC], F16, tag=f"ot{h}", name=f"ot{h}")
                        nc.vector.tensor_copy(ot[:], ops[0:64, :])
                        ots.append((h, ot))
                pending = (qc, ots, lrows, l_dram)

            # tail: the held-back proj(2) gives the PE ~13us of work while
            # the last chunk's flush chain (DVE + K=1 broadcast) resolves,
            # then proj(3)
            emit_proj(NQC - 2, list(range(C // 128)))
            emit_flush_pe(pending[0], pending[1], pending[2])
            emit_proj(NQC - 1, list(range(C // 128)))

    nc.compile()
    return nc


def _shards(W_attn, b_attn, W_proj, b_proj):
    """Per-TP-half weight shards (t = 0, 1), packed for the device layout."""
    shards = []
    for t in range(2):
        heads = list(range(t * HPC, (t + 1) * HPC))
        wqk = np.empty((C, HPC * 128), np.float16)
        bqk = np.empty(HPC * 128, np.float32)
        wv = np.empty((C, HPC * D), np.float16)
        bvv = np.empty(HPC * D, np.float16)
        # m-tile 2p = Q of head pair p (head 2p on cols 0-63 of the tile,
        # head 2p+1 on 64-127); m-tile 2p+1 = K of the same pair.
        for p in range(HPC // 2):
            ha, hb = heads[2 * p], heads[2 * p + 1]
            mq, mk = (2 * p) * 128, (2 * p + 1) * 128
            wqk[:, mq : mq + 64] = W_attn[:, ha * D : (ha + 1) * D]
            wqk[:, mq + 64 : mq + 128] = W_attn[:, hb * D : (hb + 1) * D]
            wqk[:, mk : mk + 64] = W_attn[:, C + ha * D : C + (ha + 1) * D]
            wqk[:, mk + 64 : mk + 128] = W_attn[:, C + hb * D : C + (hb + 1) * D]
            bqk[mq : mq + 64] = b_attn[ha * D : (ha + 1) * D]
            bqk[mq + 64 : mq + 128] = b_attn[hb * D : (hb + 1) * D]
            bqk[mk : mk + 64] = b_attn[C + ha * D : C + (ha + 1) * D]
            bqk[mk + 64 : mk + 128] = b_attn[C + hb * D : C + (hb + 1) * D]
        for j, h in enumerate(heads):
            wv[:, j * D : (j + 1) * D] = W_attn[:, 2 * C + h * D : 2 * C + (h + 1) * D]
            bvv[j * D : (j + 1) * D] = b_attn[2 * C + h * D : 2 * C + (h + 1) * D]
        wp = W_proj[t * HPC * D : (t + 1) * HPC * D, :].astype(np.float16)
        bpp = (b_proj if t == 0 else np.zeros_like(b_proj)).astype(np.float32)
        shards.append((wqk, bqk, wv, bvv, np.ascontiguousarray(wp), bpp))
    return shards


def _mask_np():
    kr = np.arange(128)[:, None]
    qr = np.arange(QC)[None, :]
    m = np.empty((4, 128, QC), np.float32)
    for o in range(4):
        m[o] = np.where(kr + o * 128 <= qr, 0.0, NEG)
    return m


def _in_maps(x, W_attn, b_attn, W_proj, b_proj):
    shards = _shards(W_attn, b_attn, W_proj, b_proj)
    mask = _mask_np()
    in_maps = []
    for b in range(B):
        xTb = np.ascontiguousarray(x[b].T.astype(np.float16))
        for t in range(2):
            wqk, bqk, wv, bvv, wp, bpp = shards[t]
            in_maps.append(
                {
                    "xT": xTb,
                    "wqk": wqk,
                    "bqk": bqk,
                    "wv": wv,
                    "bv": bvv,
                    "wp": wp,
                    "bp": bpp,
                    "mask": mask,
                }
            )
    return in_maps


def _gather(results):
    out = np.empty((B, T, C), np.float32)
    for b in range(B):
        acc = results[2 * b]["outT"] + results[2 * b + 1]["outT"]
        out[b] = acc.T
    return out


def kernel(x, W_attn, b_attn, W_proj, b_proj):
    x = np.asarray(x, np.float32)
    W_attn = np.asarray(W_attn, np.float32)
    b_attn = np.asarray(b_attn, np.float32)
    W_proj = np.asarray(W_proj, np.float32)
    b_proj = np.asarray(b_proj, np.float32)

    if "nc" not in _CACHE:
        _CACHE["nc"] = _build()
    nc = _CACHE["nc"]

    in_maps = _in_maps(x, W_attn, b_attn, W_proj, b_proj)
    res = run_bass_kernel_spmd(nc, in_maps, core_ids=list(range(8)))
    return _gather(res.results)

